# revision 43
# baseline (speedup 1.0000x reference)
"""Trainium2 Bass kernel for nn_AdditiveAttention (additive attention, no tanh).

Math: scores[b,q,k] = sum_h (qh[b,q,h] + kh[b,k,h]) * Wv[h]
                    = (q[b,q,:] @ (Wv@Wq)) + (k[b,k,:] @ (Wv@Wk))
                    = sq[b,q] + sk[b,k]           (rank-1 in (q,k))
softmax over k is shift-invariant, so the sq[b,q] term cancels exactly:
    attn[b,q,:] = softmax_k(mask(sk[b,:]))        (independent of q!)
    out[b,q,:]  = p[b,:] @ v[b]                   (one row, broadcast over q)

Per-core work (core i -> batch b = i//2, output half = i%2), raw Bass blocks:
    PE : sk_ps = 1*mask_row + w_eff @ kt      (one PSUM accumulation group)
         pbc_ps[:,128t:] = p_chunk.T @ ones_row  (transpose + broadcast)
         invbc_ps = ones_row.T @ inv          (1/sum broadcast to partitions)
         out_ps = pbc.T @ v                   (every row == p @ v / sum)
    ACT: p_row = exp(sk_ps)                   (no max shift; |sk| is O(5);
                                               valid_len==0 handled host-side)
    DVE: mask_row, sum, 1/sum, PSUM->SBUF copies, final scaled copy
    SP/GPSIMD: DMAs (kt 2x512KB, v 1x1MB, w4+misc on ACT queue, out 2x256KB)

Softmax normalization note: reference computes exp(s-max)/sum(exp(s-max));
we compute exp(s)/sum(exp(s)) -- identical up to fp rounding since all
unmasked s are O(5). valid_len==0 (reference: uniform over ALL positions)
is reproduced exactly by sending k=0 and an all-valid mask: p = 1/512.
"""

import numpy as np

B, LQ, LK, DQ, DK, DV, H = 4, 512, 512, 512, 512, 512, 256
NCORES = 8
NEG = -1.0e9
NT = LK // 128  # 4 k-tiles


def _build_nc():
    import concourse.bacc as bacc
    import concourse.mybir as mybir

    f32 = mybir.dt.float32
    f32r = mybir.dt.float32r
    AF = mybir.ActivationFunctionType
    OP = mybir.AluOpType
    AX = mybir.AxisListType

    nc = bacc.Bacc("TRN2", target_bir_lowering=False, debug=False,
                   num_devices=NCORES)

    # kt arrives pre-scaled by w_eff (host folds the weight vector into k.T),
    # so sk[kk] = sum_d kt[d,kk] = ones_col.T @ kt -- no [128,x] weight DMA
    # (a 128x16B strided DMA measured ~6us; everything here is contiguous).
    kt = nc.dram_tensor("kt", [DK, LK], f32r, kind="ExternalInput")
    vv = nc.dram_tensor("vv", [LK, DV], f32r, kind="ExternalInput")
    misc = nc.dram_tensor("misc", [1, LK + 1], f32, kind="ExternalInput")
    cr = nc.dram_tensor("cr", [1, 129], f32r, kind="ExternalInput")  # ones,1
    out_d = nc.dram_tensor("out", [256, DV], f32, kind="ExternalOutput")

    # HBM [512,512] row-major -> SBUF [128, NT*512]: partition p, chunk c
    # holds row c*128+p (tile c side by side in the free dim).
    kt_lo = kt[0:256, :].rearrange("(c p) d -> p c d", p=128)
    kt_hi = kt[256:512, :].rearrange("(c p) d -> p c d", p=128)
    v_all = vv[:, :].rearrange("(c p) d -> p c d", p=128)

    from contextlib import ExitStack
    with ExitStack() as es:
        kt_sb = es.enter_context(nc.sbuf_tensor("kt_sb", [128, NT * LK], f32r))
        v_sb = es.enter_context(nc.sbuf_tensor("v_sb", [128, NT * DV], f32r))
        oc_sb = es.enter_context(nc.sbuf_tensor("oc_sb", [128, 1], f32r))
        misc_sb = es.enter_context(nc.sbuf_tensor("misc_sb", [1, LK + 1], f32))
        mask_sb = es.enter_context(nc.sbuf_tensor("mask_sb", [1, LK], f32r))
        cr_sb = es.enter_context(nc.sbuf_tensor("cr_sb", [1, 129], f32r))
        ones_row = cr_sb[:, 0:128]
        one_1x1 = cr_sb[:, 128:129]
        ones_f32 = es.enter_context(nc.sbuf_tensor("ones_f32", [1, 128], f32))
        p_row = es.enter_context(nc.sbuf_tensor("p_row", [1, LK], f32r))
        ssum = es.enter_context(nc.sbuf_tensor("ssum", [1, 1], f32))
        inv_sb = es.enter_context(nc.sbuf_tensor("inv_sb", [1, 1], f32))
        invbc_sb = es.enter_context(nc.sbuf_tensor("invbc_sb", [128, 1], f32))
        pbc_sb = es.enter_context(nc.sbuf_tensor("pbc_sb", [128, NT * 128], f32r))
        ob = es.enter_context(nc.sbuf_tensor("ob", [128, DV], f32))
        oc_ps = es.enter_context(nc.psum_tensor("oc_ps", [128, 1], f32))
        sk_ps = es.enter_context(nc.psum_tensor("sk_ps", [1, LK], f32))
        pbc_ps = es.enter_context(nc.psum_tensor("pbc_ps", [128, NT * 128], f32))
        invbc_ps = es.enter_context(nc.psum_tensor("invbc_ps", [128, 1], f32))
        out_ps = es.enter_context(nc.psum_tensor("out_ps", [128, DV], f32))
        sem = lambda name: es.enter_context(nc.semaphore(name))
        s_misc, s_cr, s_kta, s_ktb, s_v, s_out = (
            sem("s_misc"), sem("s_cr"), sem("s_kta"),
            sem("s_ktb"), sem("s_v"), sem("s_out"))      # DMA sems (inc 16)
        (c_const, c_mask, c_sk, c_p, c_sum, c_inv, c_invbc_ps, c_invbc,
         c_ocps, c_oc, c_pbc_ps, c_pbc, c_out, c_ob) = (
            sem("c_const"), sem("c_mask"), sem("c_sk"), sem("c_p"),
            sem("c_sum"), sem("c_inv"), sem("c_invbc_ps"), sem("c_invbc"),
            sem("c_ocps"), sem("c_oc"),
            sem("c_pbc_ps"), sem("c_pbc"), sem("c_out"), sem("c_ob"))
        block = es.enter_context(nc.Block())

        kt_sb_lo = kt_sb[:, 0:2 * LK].rearrange("p (c d) -> p c d", d=LK)
        kt_sb_hi = kt_sb[:, 2 * LK:4 * LK].rearrange("p (c d) -> p c d", d=LK)
        v_sb_3d = v_sb[:, :].rearrange("p (c d) -> p c d", d=DV)

        @block.sync
        def _(sync):
            sync.dma_start(out=kt_sb_lo, in_=kt_lo).then_inc(s_kta, 16)
            sync.dma_start(out=kt_sb_hi, in_=kt_hi).then_inc(s_ktb, 16)
            sync.wait_ge(c_ob, 1)
            sync.dma_start(out=out_d[0:128, :], in_=ob[:, :]).then_inc(s_out, 16)
            sync.dma_start(out=out_d[128:256, :], in_=ob[:, :]).then_inc(s_out, 16)
            sync.wait_ge(s_out, 32)

        @block.scalar
        def _(scalar):
            scalar.dma_start(out=cr_sb[:, :], in_=cr[:, :]).then_inc(s_cr, 16)
            scalar.dma_start(out=misc_sb[:, :], in_=misc[:, :]).then_inc(s_misc, 16)
            # p = exp(sk + mask); act table loads while DMAs are in flight
            scalar.wait_ge(c_sk, 1)
            nc.scalar.activation(p_row[:, :], sk_ps[:, :], AF.Exp).then_inc(c_p, 1)

        @block.gpsimd
        def _(gpsimd):
            gpsimd.memset(ones_f32[:, :], 1.0).then_inc(c_const, 1)
            gpsimd.dma_start(out=v_sb_3d, in_=v_all).then_inc(s_v, 16)

        @block.vector
        def _(vector):
            # ones column for the sk reduction (from the PE transpose below)
            vector.wait_ge(c_ocps, 1)
            nc.vector.tensor_copy(oc_sb[:, :], oc_ps[:, :]).then_inc(c_oc, 1)
            # additive mask row: (iota >= valid_len) * NEG
            vector.wait_ge(s_misc, 16)
            nc.vector.tensor_scalar(out=mask_sb[:, :],
                                    in0=misc_sb[:, 0:LK],
                                    scalar1=misc_sb[:, LK:LK + 1],
                                    scalar2=NEG,
                                    op0=OP.is_ge, op1=OP.mult).then_inc(c_mask, 1)
            # sum + reciprocal (off critical path)
            vector.wait_ge(c_p, 1)
            nc.vector.tensor_reduce(out=ssum[:, :], in_=p_row[:, :],
                                    axis=AX.X, op=OP.add).then_inc(c_sum, 1)
            vector.wait_ge(c_sum, 1)
            nc.vector.reciprocal(inv_sb[:, :], ssum[:, :]).then_inc(c_inv, 1)
            vector.wait_ge(c_invbc_ps, 1)
            nc.vector.tensor_copy(invbc_sb[:, :], invbc_ps[:, :]).then_inc(c_invbc, 1)
            vector.wait_ge(c_pbc_ps, 1)
            nc.vector.tensor_copy(pbc_sb[:, :], pbc_ps[:, :]).then_inc(c_pbc, 1)
            # final scaled copy: ob = out_ps * (1/sum)
            vector.wait_ge(c_out, 1)
            vector.wait_ge(c_invbc, 1)
            nc.vector.tensor_scalar(out=ob[:, :], in0=out_ps[:, :],
                                    scalar1=invbc_sb[:, :], scalar2=None,
                                    op0=OP.mult).then_inc(c_ob, 1)

        @block.tensor
        def _(tensor):
            # build ones column [128,1] = ones_row.T @ 1 (for the sk sums)
            one_f32 = nc.const_aps.aps[(f32, 1.0)][0:1, 0:1]
            tensor.wait_ge(c_const, 1)
            nc.tensor.matmul(oc_ps[:, :], ones_f32[:, :], one_f32,
                             start=True, stop=True).then_inc(c_ocps, 1)
            # sk accumulation group: sum_d (w*kT)[d, :] + 1*mask
            tensor.wait_ge(c_oc, 1)
            tensor.wait_ge(s_kta, 16)
            nc.tensor.matmul(sk_ps[:, :], oc_sb[:, :], kt_sb[:, 0:LK],
                             start=True, stop=False)
            tensor.wait_ge(c_mask, 1)
            tensor.wait_ge(s_cr, 16)
            nc.tensor.matmul(sk_ps[:, :], one_1x1, mask_sb[:, :],
                             start=False, stop=False)
            nc.tensor.matmul(sk_ps[:, :], oc_sb[:, :], kt_sb[:, LK:2 * LK],
                             start=False, stop=False)
            tensor.wait_ge(s_ktb, 16)
            for t in (2, 3):
                mm = nc.tensor.matmul(sk_ps[:, :], oc_sb[:, :],
                                      kt_sb[:, LK * t:LK * (t + 1)],
                                      start=False, stop=(t == 3))
            mm.then_inc(c_sk, 1)
            # transpose p into partition dim, broadcast across free dim:
            # pbc_ps[:, 128t:128(t+1)] = p_chunk[1,128].T @ ones_row[1,128]
            tensor.wait_ge(c_p, 1)
            for t in range(NT):
                mm = nc.tensor.matmul(pbc_ps[:, 128 * t:128 * (t + 1)],
                                      p_row[:, 128 * t:128 * (t + 1)],
                                      ones_row, start=True, stop=True)
            mm.then_inc(c_pbc_ps, 1)
            # broadcast 1/sum to all partitions (tiny fp32 matmul)
            tensor.wait_ge(c_inv, 1)
            tensor.wait_ge(c_const, 1)
            nc.tensor.matmul(invbc_ps[:, :], ones_f32[:, :], inv_sb[:, :],
                             start=True, stop=True).then_inc(c_invbc_ps, 1)
            # out = P_bc.T @ v
            tensor.wait_ge(c_pbc, 1)
            tensor.wait_ge(s_v, 16)
            for t in range(NT):
                mm = nc.tensor.matmul(out_ps[:, :],
                                      pbc_sb[:, 128 * t:128 * (t + 1)],
                                      v_sb[:, DV * t:DV * (t + 1)],
                                      start=(t == 0), stop=(t == NT - 1))
            mm.then_inc(c_out, 1)

    nc.compile()  # Bacc register allocation + DCE
    return nc


_NC_CACHE = {}


def _get_nc():
    if "nc" not in _NC_CACHE:
        _NC_CACHE["nc"] = _build_nc()
    return _NC_CACHE["nc"]


def _round_f32r(x):
    """Round fp32 to the float32r-representable set (bf16 hi + bf16 lo),
    so the on-device fp32r matmul consumes pre-rounded data (~2^-16 rel)."""
    import ml_dtypes
    x = np.asarray(x, np.float32)
    hi = x.astype(ml_dtypes.bfloat16).astype(np.float32)
    lo = (x - hi).astype(ml_dtypes.bfloat16).astype(np.float32)
    return hi + lo


def make_in_maps(q, k, v, Wq, Wk, Wv, valid_len):
    """Host-side sharding: core i gets batch b=i//2 (k transposed, v natural),
    folded weight vector w_eff = Wv @ Wk, and [iota | valid_len] row."""
    k = np.asarray(k, dtype=np.float32)
    v = np.asarray(v, dtype=np.float32)
    Wk = np.asarray(Wk, dtype=np.float32)
    Wv = np.asarray(Wv, dtype=np.float32)
    w_eff = (Wv @ Wk)[0]  # [512]
    v = _round_f32r(v)
    iota = np.arange(LK, dtype=np.float32)
    cr = np.ones((1, 129), dtype=np.float32)
    vl = np.asarray(valid_len)
    in_maps = []
    for core in range(NCORES):
        b = core // 2
        if vl[b] > 0:
            # fold the weight vector into k.T: sk = ones.T @ (w[:,None]*k.T)
            kt_b = _round_f32r(w_eff[:, None] * k[b].T)
            vl_eff = float(vl[b])
        else:
            # reference: all positions masked -> softmax over equal values ->
            # exactly uniform 1/LK.  exp(0)/sum(exp(0)) with no mask gives the
            # same result exactly, so send k=0 and mask nothing.
            kt_b = np.zeros((DK, LK), np.float32)
            vl_eff = float(LK)
        misc = np.concatenate(
            [iota, np.array([vl_eff], dtype=np.float32)]).reshape(1, LK + 1)
        in_maps.append({
            "kt": np.ascontiguousarray(kt_b),
            "vv": np.ascontiguousarray(v[b]),
            "misc": np.ascontiguousarray(misc),
            "cr": cr,
        })
    return in_maps


def kernel(q, k, v, Wq, Wk, Wv, valid_len):
    from concourse.bass_utils import run_bass_kernel_spmd

    nc = _get_nc()
    in_maps = make_in_maps(q, k, v, Wq, Wk, Wv, valid_len)
    res = run_bass_kernel_spmd(nc, in_maps, list(range(NCORES)))
    out = np.empty((B, LQ, DV), dtype=np.float32)
    for core in range(NCORES):
        b, half = core // 2, core % 2
        out[b, 256 * half:256 * (half + 1), :] = res.results[core]["out"]
    return out


# revision 48
# speedup vs baseline: 1.1866x; 1.1866x over previous
"""Trainium2 Bass kernel for nn_AdditiveAttention (additive attention, no tanh).

Math: scores[b,q,k] = sum_h (qh[b,q,h] + kh[b,k,h]) * Wv[h]
                    = (q[b,q,:] @ (Wv@Wq)) + (k[b,k,:] @ (Wv@Wk))
                    = sq[b,q] + sk[b,k]           (rank-1 in (q,k))
softmax over k is shift-invariant, so the sq[b,q] term cancels exactly:
    attn[b,q,:] = softmax_k(mask(sk[b,:]))        (independent of q!)
    out[b,q,:]  = p[b,:] @ v[b]                   (one row, broadcast over q)

Per-core work (core i -> batch b = i//2, output half = i%2), raw Bass blocks:
    PE : sk_ps = 1*mask_row + w_eff @ kt      (one PSUM accumulation group)
         pbc_ps[:,128t:] = p_chunk.T @ ones_row  (transpose + broadcast)
         invbc_ps = ones_row.T @ inv          (1/sum broadcast to partitions)
         out_ps = pbc.T @ v                   (every row == p @ v / sum)
    ACT: p_row = exp(sk_ps)                   (no max shift; |sk| is O(5);
                                               valid_len==0 handled host-side)
    DVE: mask_row, sum, 1/sum, PSUM->SBUF copies, final scaled copy
    SP/GPSIMD: DMAs (kt 2x512KB, v 1x1MB, w4+misc on ACT queue, out 2x256KB)

Softmax normalization note: reference computes exp(s-max)/sum(exp(s-max));
we compute exp(s)/sum(exp(s)) -- identical up to fp rounding since all
unmasked s are O(5). valid_len==0 (reference: uniform over ALL positions)
is reproduced exactly by sending k=0 and an all-valid mask: p = 1/512.
"""

import numpy as np

B, LQ, LK, DQ, DK, DV, H = 4, 512, 512, 512, 512, 512, 256
NCORES = 8
NEG = -1.0e9
NT = LK // 128  # 4 k-tiles


def _build_nc():
    import concourse.bacc as bacc
    import concourse.mybir as mybir

    f32 = mybir.dt.float32
    f32r = mybir.dt.float32r
    AF = mybir.ActivationFunctionType
    OP = mybir.AluOpType
    AX = mybir.AxisListType

    nc = bacc.Bacc("TRN2", target_bir_lowering=False, debug=False,
                   num_devices=NCORES)

    # kt arrives pre-scaled by w_eff (host folds the weight vector into k.T),
    # so sk[kk] = sum_d kt[d,kk] = ones_col.T @ kt -- no [128,x] weight DMA
    # (a 128x16B strided DMA measured ~6us; everything here is contiguous).
    kt = nc.dram_tensor("kt", [DK, LK], f32r, kind="ExternalInput")
    vv = nc.dram_tensor("vv", [LK, DV], f32r, kind="ExternalInput")
    # aux row: [iota(512) | valid_len | ones(128) | 1.0]; all values are
    # exactly f32r-representable (small ints / 1.0)
    aux = nc.dram_tensor("aux", [1, LK + 130], f32r, kind="ExternalInput")
    out_d = nc.dram_tensor("out", [256, DV], f32, kind="ExternalOutput")

    # HBM [512,512] row-major -> SBUF [128, NT*512]: partition p, chunk c
    # holds row c*128+p (tile c side by side in the free dim).
    kt_lo = kt[0:256, :].rearrange("(c p) d -> p c d", p=128)
    kt_hi = kt[256:512, :].rearrange("(c p) d -> p c d", p=128)
    v_lo = vv[0:256, :].rearrange("(c p) d -> p c d", p=128)
    v_hi = vv[256:512, :].rearrange("(c p) d -> p c d", p=128)

    from contextlib import ExitStack
    with ExitStack() as es:
        kt_sb = es.enter_context(nc.sbuf_tensor("kt_sb", [128, NT * LK], f32r))
        v_sb = es.enter_context(nc.sbuf_tensor("v_sb", [128, NT * DV], f32r))
        oc_sb = es.enter_context(nc.sbuf_tensor("oc_sb", [128, 1], f32r))
        aux_sb = es.enter_context(nc.sbuf_tensor("aux_sb", [1, LK + 130], f32r))
        iota_row = aux_sb[:, 0:LK]
        vl_1x1 = aux_sb[:, LK:LK + 1].bitcast(f32)
        ones_row = aux_sb[:, LK + 1:LK + 129]
        one_1x1 = aux_sb[:, LK + 129:LK + 130]
        mask_sb = es.enter_context(nc.sbuf_tensor("mask_sb", [1, LK], f32r))
        ones_f32 = es.enter_context(nc.sbuf_tensor("ones_f32", [1, 512], f32))
        p_row = es.enter_context(nc.sbuf_tensor("p_row", [1, LK], f32r))
        ssum = es.enter_context(nc.sbuf_tensor("ssum", [1, 1], f32))
        inv_sb = es.enter_context(nc.sbuf_tensor("inv_sb", [1, 1], f32))
        invbc_sb = es.enter_context(nc.sbuf_tensor("invbc_sb", [128, 1], f32))
        pbc_sb = es.enter_context(nc.sbuf_tensor("pbc_sb", [128, NT * 128], f32r))
        ob = es.enter_context(nc.sbuf_tensor("ob", [128, DV], f32))
        oc_ps = es.enter_context(nc.psum_tensor("oc_ps", [128, 1], f32))
        sk_ps = es.enter_context(nc.psum_tensor("sk_ps", [1, LK], f32))
        # one PSUM bank per transpose tile: ACT reads tile t while PE writes
        # tile t+1 -- same-bank PE-write + engine-read is a fatal HW hazard
        pbc_ps = [es.enter_context(nc.psum_tensor(f"pbc_ps{t}", [128, 128], f32))
                  for t in range(NT)]
        invbc_ps = es.enter_context(nc.psum_tensor("invbc_ps", [128, 1], f32))
        out_ps = es.enter_context(nc.psum_tensor("out_ps", [128, DV], f32))
        sem = lambda name: es.enter_context(nc.semaphore(name))
        s_aux, s_kta, s_ktb, s_va, s_vb, s_out, s_out2 = (
            sem("s_aux"), sem("s_kta"), sem("s_ktb"), sem("s_va"), sem("s_vb"),
            sem("s_out"), sem("s_out2"))                 # DMA sems (inc 16)
        (c_const, c_mask, c_sk, c_p, c_sum, c_inv, c_invbc_ps, c_invbc,
         c_ocps, c_oc, c_tr, c_cast, c_out, c_ob) = (
            sem("c_const"), sem("c_mask"), sem("c_sk"), sem("c_p"),
            sem("c_sum"), sem("c_inv"), sem("c_invbc_ps"), sem("c_invbc"),
            sem("c_ocps"), sem("c_oc"), sem("c_tr"), sem("c_cast"),
            sem("c_out"), sem("c_ob"))
        block = es.enter_context(nc.Block())

        kt_sb_lo = kt_sb[:, 0:2 * LK].rearrange("p (c d) -> p c d", d=LK)
        kt_sb_hi = kt_sb[:, 2 * LK:4 * LK].rearrange("p (c d) -> p c d", d=LK)
        v_sb_lo = v_sb[:, 0:2 * DV].rearrange("p (c d) -> p c d", d=DV)
        v_sb_hi = v_sb[:, 2 * DV:4 * DV].rearrange("p (c d) -> p c d", d=DV)

        @block.sync
        def _(sync):
            # kt first (it gates the sk chain), then v behind it so kt gets
            # full HBM bandwidth; out rows 0:128 at the end.
            sync.dma_start(out=kt_sb_lo, in_=kt_lo).then_inc(s_kta, 16)
            sync.dma_start(out=kt_sb_hi, in_=kt_hi).then_inc(s_ktb, 16)
            sync.dma_start(out=v_sb_lo, in_=v_lo).then_inc(s_va, 16)
            sync.dma_start(out=v_sb_hi, in_=v_hi).then_inc(s_vb, 16)
            sync.wait_ge(c_ob, 1)
            sync.dma_start(out=out_d[0:128, :], in_=ob[:, :]).then_inc(s_out, 16)
            sync.wait_ge(s_out, 16)

        @block.scalar
        def _(scalar):
            scalar.dma_start(out=aux_sb[:, :], in_=aux[:, :]).then_inc(s_aux, 16)
            # p = exp(sk + mask); act table loads while DMAs are in flight
            scalar.wait_ge(c_sk, 1)
            nc.scalar.activation(p_row[:, :], sk_ps[:, :], AF.Exp).then_inc(c_p, 1)
            # per-tile PSUM->SBUF casts of P_bc (pipelines the final matmuls)
            for t in range(NT):
                scalar.wait_ge(c_tr, t + 1)
                nc.scalar.copy(pbc_sb[:, 128 * t:128 * (t + 1)],
                               pbc_ps[t][:, :]).then_inc(c_cast, 1)
            # final scaled copy: ob = out_ps * (1/sum), then out rows 128:256
            scalar.wait_ge(c_out, 1)
            scalar.wait_ge(c_invbc, 1)
            nc.scalar.mul(ob[:, :], out_ps[:, :], invbc_sb[:, :]).then_inc(c_ob, 1)
            scalar.wait_ge(c_ob, 1)
            scalar.dma_start(out=out_d[128:256, :], in_=ob[:, :]).then_inc(s_out2, 16)
            scalar.wait_ge(s_out2, 16)

        @block.gpsimd
        def _(gpsimd):
            gpsimd.memset(ones_f32[:, :], 1.0).then_inc(c_const, 1)

        @block.vector
        def _(vector):
            # ones column for the sk reduction (from the PE transpose below)
            vector.wait_ge(c_ocps, 1)
            nc.vector.tensor_copy(oc_sb[:, :], oc_ps[:, :]).then_inc(c_oc, 1)
            # additive mask row: (iota >= valid_len) * NEG
            vector.wait_ge(s_aux, 16)
            nc.vector.tensor_scalar(out=mask_sb[:, :],
                                    in0=iota_row,
                                    scalar1=vl_1x1,
                                    scalar2=NEG,
                                    op0=OP.is_ge, op1=OP.mult).then_inc(c_mask, 1)
            # sum + reciprocal (off critical path)
            vector.wait_ge(c_p, 1)
            nc.vector.tensor_reduce(out=ssum[:, :], in_=p_row[:, :],
                                    axis=AX.X, op=OP.add).then_inc(c_sum, 1)
            vector.wait_ge(c_sum, 1)
            nc.vector.reciprocal(inv_sb[:, :], ssum[:, :]).then_inc(c_inv, 1)
            vector.wait_ge(c_invbc_ps, 1)
            nc.vector.tensor_copy(invbc_sb[:, :], invbc_ps[:, :]).then_inc(c_invbc, 1)

        @block.tensor
        def _(tensor):
            # build ones column [128,1] = ones_f32.T @ 1 (for the sk sums)
            one_f32 = nc.const_aps.aps[(f32, 1.0)][0:1, 0:1]
            tensor.wait_ge(c_const, 1)
            nc.tensor.matmul(oc_ps[:, :], ones_f32[:, 0:128], one_f32,
                             start=True, stop=True).then_inc(c_ocps, 1)
            for _ in range(2):
                nc.tensor.matmul(out_ps[:, :], ones_f32[:, 0:128],
                                 ones_f32[:, :], start=True, stop=True)
            # sk accumulation group: sum_d (w*kT)[d, :] + 1*mask (mask last
            # so a late mask DMA cannot stall the kt-driven matmuls)
            tensor.wait_ge(c_oc, 1)
            tensor.wait_ge(s_kta, 16)
            for t in (0, 1):
                nc.tensor.matmul(sk_ps[:, :], oc_sb[:, :],
                                 kt_sb[:, LK * t:LK * (t + 1)],
                                 start=(t == 0), stop=False)
            tensor.wait_ge(s_ktb, 16)
            for t in (2, 3):
                nc.tensor.matmul(sk_ps[:, :], oc_sb[:, :],
                                 kt_sb[:, LK * t:LK * (t + 1)],
                                 start=False, stop=False)
            tensor.wait_ge(c_mask, 1)
            nc.tensor.matmul(sk_ps[:, :], one_1x1, mask_sb[:, :],
                             start=False, stop=True).then_inc(c_sk, 1)
            # transpose p into partition dim, broadcast across free dim:
            # pbc_ps[:, 128t:128(t+1)] = p_chunk[1,128].T @ ones_row[1,128]
            tensor.wait_ge(c_p, 1)
            for t in range(NT):
                nc.tensor.matmul(pbc_ps[t][:, :],
                                 p_row[:, 128 * t:128 * (t + 1)],
                                 ones_row, start=True, stop=True
                                 ).then_inc(c_tr, 1)
            # broadcast 1/sum to all partitions (tiny fp32 matmul)
            tensor.wait_ge(c_inv, 1)
            nc.tensor.matmul(invbc_ps[:, :], ones_f32[:, 0:128], inv_sb[:, :],
                             start=True, stop=True).then_inc(c_invbc_ps, 1)
            # out = P_bc.T @ v, pipelined per tile behind the casts
            tensor.wait_ge(s_va, 16)
            for t in range(NT):
                if t == 2:
                    tensor.wait_ge(s_vb, 16)
                tensor.wait_ge(c_cast, t + 1)
                mm = nc.tensor.matmul(out_ps[:, :],
                                      pbc_sb[:, 128 * t:128 * (t + 1)],
                                      v_sb[:, DV * t:DV * (t + 1)],
                                      start=(t == 0), stop=(t == NT - 1))
            mm.then_inc(c_out, 1)

    nc.compile()  # Bacc register allocation + DCE
    return nc


_NC_CACHE = {}


def _get_nc():
    if "nc" not in _NC_CACHE:
        _NC_CACHE["nc"] = _build_nc()
    return _NC_CACHE["nc"]


def _round_f32r(x):
    """Round fp32 to the float32r-representable set (bf16 hi + bf16 lo),
    so the on-device fp32r matmul consumes pre-rounded data (~2^-16 rel)."""
    import ml_dtypes
    x = np.asarray(x, np.float32)
    hi = x.astype(ml_dtypes.bfloat16).astype(np.float32)
    lo = (x - hi).astype(ml_dtypes.bfloat16).astype(np.float32)
    return hi + lo


def make_in_maps(q, k, v, Wq, Wk, Wv, valid_len):
    """Host-side sharding: core i gets batch b=i//2 (k transposed, v natural),
    folded weight vector w_eff = Wv @ Wk, and [iota | valid_len] row."""
    k = np.asarray(k, dtype=np.float32)
    v = np.asarray(v, dtype=np.float32)
    Wk = np.asarray(Wk, dtype=np.float32)
    Wv = np.asarray(Wv, dtype=np.float32)
    w_eff = (Wv @ Wk)[0]  # [512]
    v = _round_f32r(v)
    iota = np.arange(LK, dtype=np.float32)
    ones129 = np.ones(129, dtype=np.float32)
    vl = np.asarray(valid_len)
    in_maps = []
    for core in range(NCORES):
        b = core // 2
        if vl[b] > 0:
            # fold the weight vector into k.T: sk = ones.T @ (w[:,None]*k.T)
            kt_b = _round_f32r(w_eff[:, None] * k[b].T)
            vl_eff = float(vl[b])
        else:
            # reference: all positions masked -> softmax over equal values ->
            # exactly uniform 1/LK.  exp(0)/sum(exp(0)) with no mask gives the
            # same result exactly, so send k=0 and mask nothing.
            kt_b = np.zeros((DK, LK), np.float32)
            vl_eff = float(LK)
        aux = np.concatenate(
            [iota, np.array([vl_eff], dtype=np.float32), ones129]
        ).reshape(1, LK + 130)
        in_maps.append({
            "kt": np.ascontiguousarray(kt_b),
            "vv": np.ascontiguousarray(v[b]),
            "aux": np.ascontiguousarray(aux),
        })
    return in_maps


def kernel(q, k, v, Wq, Wk, Wv, valid_len):
    from concourse.bass_utils import run_bass_kernel_spmd

    nc = _get_nc()
    in_maps = make_in_maps(q, k, v, Wq, Wk, Wv, valid_len)
    res = run_bass_kernel_spmd(nc, in_maps, list(range(NCORES)))
    out = np.empty((B, LQ, DV), dtype=np.float32)
    for core in range(NCORES):
        b, half = core // 2, core % 2
        out[b, 256 * half:256 * (half + 1), :] = res.results[core]["out"]
    return out


# revision 49
# speedup vs baseline: 1.3480x; 1.1360x over previous
"""Trainium2 Bass kernel for nn_AdditiveAttention (additive attention, no tanh).

Math: scores[b,q,k] = sum_h (qh[b,q,h] + kh[b,k,h]) * Wv[h]
                    = (q[b,q,:] @ (Wv@Wq)) + (k[b,k,:] @ (Wv@Wk))
                    = sq[b,q] + sk[b,k]           (rank-1 in (q,k))
softmax over k is shift-invariant, so the sq[b,q] term cancels exactly:
    attn[b,q,:] = softmax_k(mask(sk[b,:]))        (independent of q!)
    out[b,q,:]  = p[b,:] @ v[b]                   (one row, broadcast over q)

Per-core work (core i -> batch b = i//2, output half = i%2), raw Bass blocks:
    PE : sk_ps = 1*mask_row + w_eff @ kt      (one PSUM accumulation group)
         pbc_ps[:,128t:] = p_chunk.T @ ones_row  (transpose + broadcast)
         invbc_ps = ones_row.T @ inv          (1/sum broadcast to partitions)
         out_ps = pbc.T @ v                   (every row == p @ v / sum)
    ACT: p_row = exp(sk_ps)                   (no max shift; |sk| is O(5);
                                               valid_len==0 handled host-side)
    DVE: mask_row, sum, 1/sum, PSUM->SBUF copies, final scaled copy
    SP/GPSIMD: DMAs (kt 2x512KB, v 1x1MB, w4+misc on ACT queue, out 2x256KB)

Softmax normalization note: reference computes exp(s-max)/sum(exp(s-max));
we compute exp(s)/sum(exp(s)) -- identical up to fp rounding since all
unmasked s are O(5). valid_len==0 (reference: uniform over ALL positions)
is reproduced exactly by sending k=0 and an all-valid mask: p = 1/512.
"""

import numpy as np

B, LQ, LK, DQ, DK, DV, H = 4, 512, 512, 512, 512, 512, 256
NCORES = 8
NEG = -1.0e9
NT = LK // 128  # 4 k-tiles


def _build_nc():
    import concourse.bacc as bacc
    import concourse.mybir as mybir

    f32 = mybir.dt.float32
    f32r = mybir.dt.float32r
    AF = mybir.ActivationFunctionType
    OP = mybir.AluOpType
    AX = mybir.AxisListType

    nc = bacc.Bacc("TRN2", target_bir_lowering=False, debug=False,
                   num_devices=NCORES)

    # kt arrives pre-scaled by w_eff (host folds the weight vector into k.T),
    # so sk[kk] = sum_d kt[d,kk] = ones_col.T @ kt -- no [128,x] weight DMA
    # (a 128x16B strided DMA measured ~6us; everything here is contiguous).
    kt = nc.dram_tensor("kt", [DK, LK], f32r, kind="ExternalInput")
    vv = nc.dram_tensor("vv", [LK, DV], f32r, kind="ExternalInput")
    # aux row: [iota(512) | valid_len | ones(128) | 1.0]; all values are
    # exactly f32r-representable (small ints / 1.0)
    aux = nc.dram_tensor("aux", [1, LK + 130], f32r, kind="ExternalInput")
    out_d = nc.dram_tensor("out", [256, DV], f32, kind="ExternalOutput")

    # HBM [512,512] row-major -> SBUF [128, NT*512]: partition p, chunk c
    # holds row c*128+p (tile c side by side in the free dim).
    kt_lo = kt[0:256, :].rearrange("(c p) d -> p c d", p=128)
    kt_hi = kt[256:512, :].rearrange("(c p) d -> p c d", p=128)
    v_lo = vv[0:256, :].rearrange("(c p) d -> p c d", p=128)
    v_hi = vv[256:512, :].rearrange("(c p) d -> p c d", p=128)

    from contextlib import ExitStack
    with ExitStack() as es:
        kt_sb = es.enter_context(nc.sbuf_tensor("kt_sb", [128, NT * LK], f32r))
        v_sb = es.enter_context(nc.sbuf_tensor("v_sb", [128, NT * DV], f32r))
        oc_sb = es.enter_context(nc.sbuf_tensor("oc_sb", [128, 1], f32r))
        aux_sb = es.enter_context(nc.sbuf_tensor("aux_sb", [1, LK + 130], f32r))
        iota_row = aux_sb[:, 0:LK]
        vl_1x1 = aux_sb[:, LK:LK + 1].bitcast(f32)
        ones_row = aux_sb[:, LK + 1:LK + 129]
        one_1x1 = aux_sb[:, LK + 129:LK + 130]
        mask_sb = es.enter_context(nc.sbuf_tensor("mask_sb", [1, LK], f32r))
        ones_f32 = es.enter_context(nc.sbuf_tensor("ones_f32", [1, 512], f32))
        p_row = es.enter_context(nc.sbuf_tensor("p_row", [1, LK], f32r))
        ssum = es.enter_context(nc.sbuf_tensor("ssum", [1, 1], f32))
        inv_sb = es.enter_context(nc.sbuf_tensor("inv_sb", [1, 1], f32))
        invbc_sb = es.enter_context(nc.sbuf_tensor("invbc_sb", [128, 1], f32))
        pbc_sb = es.enter_context(nc.sbuf_tensor("pbc_sb", [128, NT * 128], f32r))
        ob = es.enter_context(nc.sbuf_tensor("ob", [128, DV], f32))
        oc_ps = es.enter_context(nc.psum_tensor("oc_ps", [128, 1], f32))
        sk_ps = es.enter_context(nc.psum_tensor("sk_ps", [1, LK], f32))
        # one PSUM bank per transpose tile: ACT reads tile t while PE writes
        # tile t+1 -- same-bank PE-write + engine-read is a fatal HW hazard
        pbc_ps = [es.enter_context(nc.psum_tensor(f"pbc_ps{t}", [128, 128], f32))
                  for t in range(NT)]
        invbc_ps = es.enter_context(nc.psum_tensor("invbc_ps", [128, 1], f32))
        out_ps = es.enter_context(nc.psum_tensor("out_ps", [128, DV], f32))
        sem = lambda name: es.enter_context(nc.semaphore(name))
        s_aux, s_kta, s_ktb, s_va, s_vb, s_out, s_out2 = (
            sem("s_aux"), sem("s_kta"), sem("s_ktb"), sem("s_va"), sem("s_vb"),
            sem("s_out"), sem("s_out2"))                 # DMA sems (inc 16)
        (c_const, c_mask, c_sk, c_p, c_sum, c_inv, c_invbc_ps, c_invbc,
         c_ocps, c_oc, c_tr, c_cast, c_out, c_ob) = (
            sem("c_const"), sem("c_mask"), sem("c_sk"), sem("c_p"),
            sem("c_sum"), sem("c_inv"), sem("c_invbc_ps"), sem("c_invbc"),
            sem("c_ocps"), sem("c_oc"), sem("c_tr"), sem("c_cast"),
            sem("c_out"), sem("c_ob"))
        block = es.enter_context(nc.Block())

        kt_sb_lo = kt_sb[:, 0:2 * LK].rearrange("p (c d) -> p c d", d=LK)
        kt_sb_hi = kt_sb[:, 2 * LK:4 * LK].rearrange("p (c d) -> p c d", d=LK)
        v_sb_lo = v_sb[:, 0:2 * DV].rearrange("p (c d) -> p c d", d=DV)
        v_sb_hi = v_sb[:, 2 * DV:4 * DV].rearrange("p (c d) -> p c d", d=DV)

        @block.sync
        def _(sync):
            # kt first (it gates the sk chain), then v behind it so kt gets
            # full HBM bandwidth; out rows 0:128 at the end.
            sync.dma_start(out=kt_sb_lo, in_=kt_lo).then_inc(s_kta, 16)
            sync.dma_start(out=kt_sb_hi, in_=kt_hi).then_inc(s_ktb, 16)
            sync.wait_ge(s_kta, 16)
            sync.dma_start(out=v_sb_lo, in_=v_lo).then_inc(s_va, 16)
            sync.dma_start(out=v_sb_hi, in_=v_hi).then_inc(s_vb, 16)
            sync.wait_ge(c_ob, 1)
            sync.dma_start(out=out_d[0:128, :], in_=ob[:, :]).then_inc(s_out, 16)
            sync.wait_ge(s_out, 16)

        @block.scalar
        def _(scalar):
            scalar.dma_start(out=aux_sb[:, :], in_=aux[:, :]).then_inc(s_aux, 16)
            # p = exp(sk + mask); act table loads while DMAs are in flight
            scalar.wait_ge(c_sk, 1)
            nc.scalar.activation(p_row[:, 0:256], sk_ps[:, 0:256],
                                 AF.Exp).then_inc(c_p, 1)
            nc.scalar.activation(p_row[:, 256:512], sk_ps[:, 256:512],
                                 AF.Exp).then_inc(c_p, 1)
            # per-tile PSUM->SBUF casts of P_bc (pipelines the final matmuls)
            for t in range(NT):
                scalar.wait_ge(c_tr, t + 1)
                nc.scalar.copy(pbc_sb[:, 128 * t:128 * (t + 1)],
                               pbc_ps[t][:, :]).then_inc(c_cast, 1)
            # final scaled copy: ob = out_ps * (1/sum), then out rows 128:256
            scalar.wait_ge(c_out, 1)
            scalar.wait_ge(c_invbc, 1)
            nc.scalar.mul(ob[:, :], out_ps[:, :], invbc_sb[:, :]).then_inc(c_ob, 1)
            scalar.wait_ge(c_ob, 1)
            scalar.dma_start(out=out_d[128:256, :], in_=ob[:, :]).then_inc(s_out2, 16)
            scalar.wait_ge(s_out2, 16)

        @block.gpsimd
        def _(gpsimd):
            gpsimd.memset(ones_f32[:, :], 1.0).then_inc(c_const, 1)

        @block.vector
        def _(vector):
            # ones column for the sk reduction (from the PE transpose below)
            vector.wait_ge(c_ocps, 1)
            nc.vector.tensor_copy(oc_sb[:, :], oc_ps[:, :]).then_inc(c_oc, 1)
            # additive mask row: (iota >= valid_len) * NEG
            vector.wait_ge(s_aux, 16)
            nc.vector.tensor_scalar(out=mask_sb[:, :],
                                    in0=iota_row,
                                    scalar1=vl_1x1,
                                    scalar2=NEG,
                                    op0=OP.is_ge, op1=OP.mult).then_inc(c_mask, 1)
            # sum + reciprocal (off critical path)
            vector.wait_ge(c_p, 2)
            nc.vector.tensor_reduce(out=ssum[:, :], in_=p_row[:, :],
                                    axis=AX.X, op=OP.add).then_inc(c_sum, 1)
            vector.wait_ge(c_sum, 1)
            nc.vector.reciprocal(inv_sb[:, :], ssum[:, :]).then_inc(c_inv, 1)
            vector.wait_ge(c_invbc_ps, 1)
            nc.vector.tensor_copy(invbc_sb[:, :], invbc_ps[:, :]).then_inc(c_invbc, 1)

        @block.tensor
        def _(tensor):
            # build ones column [128,1] = ones_f32.T @ 1 (for the sk sums)
            one_f32 = nc.const_aps.aps[(f32, 1.0)][0:1, 0:1]
            tensor.wait_ge(c_const, 1)
            nc.tensor.matmul(oc_ps[:, :], ones_f32[:, 0:128], one_f32,
                             start=True, stop=True).then_inc(c_ocps, 1)
            # sk accumulation group: sum_d (w*kT)[d, :] + 1*mask (mask last
            # so a late mask DMA cannot stall the kt-driven matmuls)
            tensor.wait_ge(c_oc, 1)
            tensor.wait_ge(s_kta, 16)
            for t in (0, 1):
                nc.tensor.matmul(sk_ps[:, :], oc_sb[:, :],
                                 kt_sb[:, LK * t:LK * (t + 1)],
                                 start=(t == 0), stop=False)
            tensor.wait_ge(s_ktb, 16)
            for t in (2, 3):
                nc.tensor.matmul(sk_ps[:, :], oc_sb[:, :],
                                 kt_sb[:, LK * t:LK * (t + 1)],
                                 start=False, stop=False)
            tensor.wait_ge(c_mask, 1)
            nc.tensor.matmul(sk_ps[:, :], one_1x1, mask_sb[:, :],
                             start=False, stop=True).then_inc(c_sk, 1)
            # transpose p into partition dim, broadcast across free dim:
            # pbc_ps[:, 128t:128(t+1)] = p_chunk[1,128].T @ ones_row[1,128]
            for t in range(NT):
                tensor.wait_ge(c_p, 1 if t < 2 else 2)
                nc.tensor.matmul(pbc_ps[t][:, :],
                                 p_row[:, 128 * t:128 * (t + 1)],
                                 ones_row, start=True, stop=True
                                 ).then_inc(c_tr, 1)
            # broadcast 1/sum to all partitions (tiny fp32 matmul)
            tensor.wait_ge(c_inv, 1)
            nc.tensor.matmul(invbc_ps[:, :], ones_f32[:, 0:128], inv_sb[:, :],
                             start=True, stop=True).then_inc(c_invbc_ps, 1)
            # out = P_bc.T @ v, pipelined per tile behind the casts
            tensor.wait_ge(s_va, 16)
            for t in range(NT):
                if t == 2:
                    tensor.wait_ge(s_vb, 16)
                tensor.wait_ge(c_cast, t + 1)
                mm = nc.tensor.matmul(out_ps[:, :],
                                      pbc_sb[:, 128 * t:128 * (t + 1)],
                                      v_sb[:, DV * t:DV * (t + 1)],
                                      start=(t == 0), stop=(t == NT - 1))
            mm.then_inc(c_out, 1)

    nc.compile()  # Bacc register allocation + DCE
    return nc


_NC_CACHE = {}


def _get_nc():
    if "nc" not in _NC_CACHE:
        _NC_CACHE["nc"] = _build_nc()
    return _NC_CACHE["nc"]


def _round_f32r(x):
    """Round fp32 to the float32r-representable set (bf16 hi + bf16 lo),
    so the on-device fp32r matmul consumes pre-rounded data (~2^-16 rel)."""
    import ml_dtypes
    x = np.asarray(x, np.float32)
    hi = x.astype(ml_dtypes.bfloat16).astype(np.float32)
    lo = (x - hi).astype(ml_dtypes.bfloat16).astype(np.float32)
    return hi + lo


def make_in_maps(q, k, v, Wq, Wk, Wv, valid_len):
    """Host-side sharding: core i gets batch b=i//2 (k transposed, v natural),
    folded weight vector w_eff = Wv @ Wk, and [iota | valid_len] row."""
    k = np.asarray(k, dtype=np.float32)
    v = np.asarray(v, dtype=np.float32)
    Wk = np.asarray(Wk, dtype=np.float32)
    Wv = np.asarray(Wv, dtype=np.float32)
    w_eff = (Wv @ Wk)[0]  # [512]
    v = _round_f32r(v)
    iota = np.arange(LK, dtype=np.float32)
    ones129 = np.ones(129, dtype=np.float32)
    vl = np.asarray(valid_len)
    in_maps = []
    for core in range(NCORES):
        b = core // 2
        if vl[b] > 0:
            # fold the weight vector into k.T: sk = ones.T @ (w[:,None]*k.T)
            kt_b = _round_f32r(w_eff[:, None] * k[b].T)
            vl_eff = float(vl[b])
        else:
            # reference: all positions masked -> softmax over equal values ->
            # exactly uniform 1/LK.  exp(0)/sum(exp(0)) with no mask gives the
            # same result exactly, so send k=0 and mask nothing.
            kt_b = np.zeros((DK, LK), np.float32)
            vl_eff = float(LK)
        aux = np.concatenate(
            [iota, np.array([vl_eff], dtype=np.float32), ones129]
        ).reshape(1, LK + 130)
        in_maps.append({
            "kt": np.ascontiguousarray(kt_b),
            "vv": np.ascontiguousarray(v[b]),
            "aux": np.ascontiguousarray(aux),
        })
    return in_maps


def kernel(q, k, v, Wq, Wk, Wv, valid_len):
    from concourse.bass_utils import run_bass_kernel_spmd

    nc = _get_nc()
    in_maps = make_in_maps(q, k, v, Wq, Wk, Wv, valid_len)
    res = run_bass_kernel_spmd(nc, in_maps, list(range(NCORES)))
    out = np.empty((B, LQ, DV), dtype=np.float32)
    for core in range(NCORES):
        b, half = core // 2, core % 2
        out[b, 256 * half:256 * (half + 1), :] = res.results[core]["out"]
    return out
